# revision 11
# baseline (speedup 1.0000x reference)
"""Trainium2 Bass kernel for nn_CFTAOBlock2D (v3).

Sharding: pure data-parallel over (batch b, channel-half) -> 8 cores.
Each core gets its batch image with channels PERMUTED so its own 32 output
channels are always [0:32] (SPMD: one program, per-core data differs).

v3 vs v2 (281us):
  - host ships x in THREE layouts (c-major fp16, h-major fp16, and the
    padded own-channel (q,c)-layout) -> all loads are contiguous, no
    on-device memsets, tiny DMA descriptor counts.
  - consts packed into 5 params (was 21 DMAs).
  - 6 of 8 dw taps on PE (diag-matmul accumulate into z PSUM); the 2
    horizontal taps stay on DVE (stt is always 1x; fewer is better).
  - bconst folded into the y W-inverse matmul via a 65th contraction row.
  - S6 processed in fc-pairs: 1 gelu [128,4096], adds split DVE/gpsimd,
    out-DMA issue split across sync+scalar queues.
"""
from contextlib import ExitStack

import numpy as np

import concourse.bass as bass
import concourse.bacc as bacc
import concourse.tile as tile
from concourse import mybir
from concourse.bass_utils import run_bass_kernel_spmd

F32 = mybir.dt.float32
FP16 = mybir.dt.float16
AX = mybir.AluOpType
AF = mybir.ActivationFunctionType

B, C, H, W = 4, 64, 256, 256
M1, M2 = 32, 32
HALF_M = 16
LSEG, RADIAL_K = 4, 4
LOCAL_SCALE, SPATIAL_SCALE, SPEC_SCALE = 0.3, 0.15, 1.0
HW = H * W
NQ, QH = 4, 64
FQ = QH * W           # 16384
NFC = 8
FC = FQ // NFC        # 2048
OC = 32
N_CORES = 8

RS = W + 2            # 258 padded row stride
BB = 4
NROW = QH + 2         # 66 rows (2 halo)
XBLEN = 17040         # BB + NROW*RS = 17032, padded to mult of 8

PE_TAPS = [(-1, -1), (-1, 0), (-1, 1), (1, -1), (1, 0), (1, 1)]
DVE_TAPS = [(0, -1), (0, 1)]


# ---------------------------------------------------------------- host math
def _softplus(x):
    x = np.asarray(x, np.float64)
    return np.log1p(np.exp(-np.abs(x))) + np.maximum(x, 0.0)


def _softmax(x):
    e = np.exp(np.asarray(x, np.float64) - np.max(x))
    return e / e.sum()


def _modal_multiplier(f):
    gh = _softmax(f["seg_h_h"]) * LSEG
    gw = _softmax(f["seg_h_w"]) * LSEG
    seg_r = (np.arange(M1) * LSEG) // M1
    seg_c = (np.arange(M2) * LSEG) // M2
    seg_gain = gh[seg_r][:, None] * gw[seg_c][None, :]

    ky = np.linspace(0.0, 1.0, M1)
    kx = np.linspace(0.0, 1.0, M2)
    Ky, Kx = ky[:, None], kx[None, :]
    r2 = Ky * Ky + Kx * Kx
    r = np.sqrt(r2 + 1e-12)
    nu0 = _softplus(f["nu_log"])
    alpha0 = _softplus(f["alpha_log"])
    c_amp = _softplus(f["c_log"])
    amp_base = np.exp(-nu0 * r2) + c_amp / (1.0 + alpha0 * r2 + 1e-6)
    w0 = (r <= 0.33).astype(np.float64)
    w2b = (r >= 0.66).astype(np.float64)
    w1 = np.maximum(1.0 - w0 - w2b, 0.0)
    g = _softplus(f["band_gain"])
    amp_base = amp_base * ((1.0 + g[0]) * w0 + (1.0 + g[1]) * w1 + (1.0 + g[2]) * w2b)
    phi_base = np.float64(f["omega_y"]) * Ky + np.float64(f["omega_x"]) * Kx

    B_rad = np.stack([r**k for k in range(RADIAL_K)], axis=0)
    amp_delta = np.einsum("ck,khw->chw", _softplus(f["amp_coef"]), B_rad)
    phase_delta = np.einsum(
        "ck,khw->chw", np.asarray(f["phase_coef"], np.float64), B_rad)
    amp_full = amp_base[None] * (1.0 + np.maximum(amp_delta, 0.0))
    phi_full = phi_base[None] + phase_delta
    kernel = (np.cos(phi_full) + 1j * np.sin(phi_full)) * amp_full
    fk = (np.asarray(f["free_kernel_re"], np.float64)
          + 1j * np.asarray(f["free_kernel_im"], np.float64))
    return seg_gain[None] * kernel * SPEC_SCALE * (1.0 + np.float64(f["free_eps"]) * fk)


def _dft_mats():
    hh = np.arange(H)
    fr = np.concatenate([np.arange(HALF_M), np.arange(H - (M1 - HALF_M), H)])
    ang_h = 2.0 * np.pi * np.outer(hh, fr) / H
    fhT = np.concatenate([np.cos(ang_h), -np.sin(ang_h)], axis=1)  # (256, 64)

    ww = np.arange(W)
    mm = np.arange(M2)
    ang_w = 2.0 * np.pi * np.outer(ww, mm) / W
    FwR, FwI = np.cos(ang_w), -np.sin(ang_w)
    fwA = np.concatenate([FwR, FwI], axis=1)            # (256, 64)
    fwB = np.concatenate([-FwI, FwR], axis=1)           # (256, 64)

    GhR = np.cos(ang_h).T / H
    GhI = np.sin(ang_h).T / H
    ghR2 = np.concatenate([GhR, -GhI], axis=0)          # (64, 256)
    ghI2 = np.concatenate([GhI, GhR], axis=0)           # (64, 256)
    cm = np.full(M2, 2.0)
    cm[0] = 1.0
    GwR = (cm[:, None] * np.cos(ang_w.T)) / W
    GwI = (cm[:, None] * np.sin(ang_w.T)) / W
    gw2 = np.concatenate([GwR, -GwI], axis=0)           # (64, 256)
    return fhT, fwA, fwB, ghR2, ghI2, gw2


def _per_core_inputs(inputs):
    f = {k: np.asarray(v) for k, v in inputs.items()}
    x = np.asarray(f["x"], np.float32)
    Mc = _modal_multiplier(f)
    fhT, fwA, fwB, ghR2, ghI2, gw2 = _dft_mats()

    kd = (SPATIAL_SCALE * np.asarray(f["w_dw3"], np.float64)[:, 0])  # (64, 3, 3)
    w_local = np.asarray(f["w_local"], np.float64)
    w_mlp1 = np.asarray(f["w_mlp1"], np.float64)
    w_mlp2 = np.asarray(f["w_mlp2"], np.float64)
    b_local = np.asarray(f["b_local"], np.float64)
    b_dw3 = np.asarray(f["b_dw3"], np.float64)
    b_mlp1 = np.asarray(f["b_mlp1"], np.float64)
    b_mlp2 = np.asarray(f["b_mlp2"], np.float64)
    gamma = np.asarray(f["gamma"], np.float64)
    beta = np.asarray(f["beta"], np.float64)

    qones = np.zeros((128, 32), np.float32)
    for p in range(128):
        qones[p, p % 32] = 1.0
    qonesT = np.ascontiguousarray(qones.T)
    ident = np.eye(128, dtype=np.float32)

    in_maps = []
    for core in range(N_CORES):
        b, half = core // 2, core % 2
        perm = np.concatenate([np.arange(half * 32, half * 32 + 32),
                               np.arange((1 - half) * 32, (1 - half) * 32 + 32)])
        xbv16 = x[b][perm].astype(np.float16)           # (64, 256, 256)
        oc = perm[:OC]

        # h-major copy for the spectral H-DFT
        xhh = np.ascontiguousarray(xbv16.transpose(1, 0, 2))  # (256, 64, 256)[h,c,w]
        xhh = xhh[:, :OC, :]                                  # own channels only

        # padded own-channel (q,c) layout with halos, pads pre-zeroed
        xbp = np.zeros((128, XBLEN), np.float16)
        xo = xbv16[:OC].reshape(OC, H, W)
        for q in range(NQ):
            r0 = 64 * q - 1
            for s in range(NROW):
                r = r0 + s
                if r < 0 or r >= H:
                    continue
                xbp[32 * q:32 * q + 32, BB + s * RS + 2:BB + s * RS + 2 + W] = xo[:, r]

        mcR = np.empty((32, 2, 16, 32), np.float32)
        mcI = np.empty((32, 2, 16, 32), np.float32)
        for ci in range(OC):
            par, pair = ci % 2, ci // 2
            mcR[:, par, pair, :] = Mc[oc[ci]].real.astype(np.float32)
            mcI[:, par, pair, :] = Mc[oc[ci]].imag.astype(np.float32)

        wlocT = (LOCAL_SCALE * w_local[oc][:, perm].T)
        for ci in range(OC):
            wlocT[ci, ci] += kd[oc[ci], 1, 1]
        wlocT2 = np.zeros((128, 64), np.float64)
        wlocT2[0:64, 0:32] = wlocT
        wlocT2[64:128, 32:64] = wlocT
        wm1T = w_mlp1[:, perm].T
        wm1T2 = np.concatenate([wm1T, wm1T], axis=0)    # (128, 128)
        wm2T = w_mlp2[oc].T                             # (128, 32)

        ktaps = np.zeros((128, len(DVE_TAPS)), np.float32)
        for t, (dy, dx) in enumerate(DVE_TAPS):
            ktaps[:, t] = np.tile(kd[oc, dy + 1, dx + 1], NQ)
        kdiag = np.zeros((128, len(PE_TAPS), 128), np.float32)
        for t, (dy, dx) in enumerate(PE_TAPS):
            kv = np.tile(kd[oc, dy + 1, dx + 1], NQ)
            kdiag[np.arange(128), t, np.arange(128)] = kv

        bconst = (LOCAL_SCALE * b_local[oc] + SPATIAL_SCALE * b_dw3[oc] + b_mlp2[oc])
        bconst128 = np.tile(bconst, NQ)                 # (128,) in (q,c) order
        # 65th-row trick: bcrow goes into Zh2e[64]; gw2e row 64 is ones
        bcrow = np.broadcast_to(
            bconst128.reshape(1, NQ, 32, 1), (1, NQ, 32, 64)).astype(np.float16)

        # ---- packed consts ----
        ckA = np.concatenate([                                   # [128, *] fp16
            fhT.reshape(2, 128, 64).transpose(1, 0, 2).reshape(128, 128),
            fwA.reshape(2, 128, 64).transpose(1, 0, 2).reshape(128, 128),
            fwB.reshape(2, 128, 64).transpose(1, 0, 2).reshape(128, 128),
            wm1T2, wlocT2, wm2T,
            kdiag.reshape(128, len(PE_TAPS) * 128),
        ], axis=1).astype(np.float16)
        ckB = np.concatenate([ident, qones, ktaps,
                              b_mlp1.reshape(128, 1)], axis=1).astype(np.float32)
        gw2e = np.concatenate([gw2, np.ones((1, 256))], axis=0)  # (65, 256)
        ckC = np.zeros((65, 768), np.float16)
        ckC[0:64, 0:256] = ghR2.astype(np.float16)
        ckC[0:64, 256:512] = ghI2.astype(np.float16)
        ckC[:, 512:768] = gw2e.astype(np.float16)
        ckD = np.concatenate([
            qonesT, gamma[oc].reshape(32, 1), beta[oc].reshape(32, 1),
            mcR.reshape(32, 1024), mcI.reshape(32, 1024)], axis=1).astype(np.float32)

        in_maps.append({
            "xbh": np.ascontiguousarray(xbv16),
            "xhh": np.ascontiguousarray(xhh),
            "xbp": xbp,
            "bcrow": np.ascontiguousarray(bcrow),
            "ckA": np.ascontiguousarray(ckA),
            "ckB": np.ascontiguousarray(ckB),
            "ckC": np.ascontiguousarray(ckC),
            "ckD": np.ascontiguousarray(ckD),
        })
    return in_maps


CKA_W = 128 + 128 + 128 + 128 + 64 + 32 + len(PE_TAPS) * 128
CKB_W = 128 + 32 + len(DVE_TAPS) + 1
CKD_W = 128 + 1 + 1 + 1024 + 1024


# ---------------------------------------------------------------- device code
def _build_program():
    nc = bacc.Bacc(None, target_bir_lowering=False, debug=False)
    P = {}

    def di(name, shape, dtype=F32):
        P[name] = nc.declare_dram_parameter(name, list(shape), dtype, isOutput=False)

    di("xbh", (C, H, W), FP16)
    di("xhh", (H, OC, W), FP16)
    di("xbp", (128, XBLEN), FP16)
    di("bcrow", (1, NQ, 32, 64), FP16)
    di("ckA", (128, CKA_W), FP16)
    di("ckB", (128, CKB_W), F32)
    di("ckC", (65, 768), FP16)
    di("ckD", (32, CKD_W), F32)
    outp = nc.declare_dram_parameter("outp", [OC, H, W], F32, isOutput=True)

    with tile.TileContext(nc) as tc, ExitStack() as ctx:
        _body(ctx, tc, P, outp)
    nc.finalize()
    return nc


def _body(ctx, tc, P, outp):
    nc = tc.nc
    xbh = P["xbh"]
    xbh_f = xbh.rearrange("c h w -> c (h w)")                    # (64, 65536) fp16
    out_f = outp.rearrange("c h w -> c (h w)")                   # (32, 65536) f32

    consts = ctx.enter_context(tc.tile_pool(name="consts", bufs=1))
    ckA = consts.tile([128, CKA_W], FP16, tag="ckA")
    nc.sync.dma_start(out=ckA, in_=P["ckA"][:])
    ckB = consts.tile([128, CKB_W], F32, tag="ckB")
    nc.sync.dma_start(out=ckB, in_=P["ckB"][:])
    ckC = consts.tile([65, 768], FP16, tag="ckC")
    nc.sync.dma_start(out=ckC, in_=P["ckC"][:])
    ckD = consts.tile([32, CKD_W], F32, tag="ckD")
    nc.sync.dma_start(out=ckD, in_=P["ckD"][:])

    o = 0
    fhT_s = ckA[:, o:o + 128].rearrange("p (t m) -> p t m", t=2); o += 128
    fwA_s = ckA[:, o:o + 128].rearrange("p (t m) -> p t m", t=2); o += 128
    fwB_s = ckA[:, o:o + 128].rearrange("p (t m) -> p t m", t=2); o += 128
    wm1T2_s = ckA[:, o:o + 128]; o += 128
    wlocT2_s = ckA[:, o:o + 64]; o += 64
    wm2T_s = ckA[:, o:o + 32]; o += 32
    kdiag_s = ckA[:, o:o + len(PE_TAPS) * 128].rearrange(
        "p (t m) -> p t m", t=len(PE_TAPS)); o += len(PE_TAPS) * 128

    o = 0
    ident_s = ckB[:, o:o + 128]; o += 128
    qones_s = ckB[:, o:o + 32]; o += 32
    ktaps_s = ckB[:, o:o + len(DVE_TAPS)]; o += len(DVE_TAPS)
    bm1_s = ckB[:, o:o + 1]; o += 1

    ghR2_s = ckC[0:64, 0:256]
    ghI2_s = ckC[0:64, 256:512]
    gw2e_s = ckC[:, 512:768]                                     # (65, 256)

    o = 0
    qonesT_s = ckD[:, o:o + 128]; o += 128
    gam_s = ckD[:, o:o + 1]; o += 1
    bet_s = ckD[:, o:o + 1]; o += 1
    mcR_s = ckD[:, o:o + 1024].rearrange("p (a b c) -> p a b c", a=2, b=16); o += 1024
    mcI_s = ckD[:, o:o + 1024].rearrange("p (a b c) -> p a b c", a=2, b=16); o += 1024

    mid = ctx.enter_context(tc.tile_pool(name="mid", bufs=1))
    Qst = mid.tile([64, 16, 2, 32], FP16, tag="Qst")
    Qst_c = Qst.rearrange("p a b w -> p (a b) w")                # (64, 32, 32)
    # Zh2e: 65 x (q, c, hl); row 64 = bconst (bcrow), pairs with gw2e ones row
    Zh2e = mid.tile([65, 4, 32, 64], FP16, tag="Zh2e")
    nc.sync.dma_start(out=Zh2e[64:65], in_=P["bcrow"][:])

    main = ctx.enter_context(tc.tile_pool(name="main", bufs=1))
    xbp = main.tile([128, XBLEN], FP16, tag="xbp")
    nc.scalar.dma_start(out=xbp, in_=P["xbp"][:])
    zbuf = main.tile([128, FQ], FP16, tag="zbuf")
    szc = main.tile([128, 2 * NFC], F32, tag="szc")
    sqc = main.tile([128, NFC], F32, tag="sqc")

    # ---------------- S1 + S2: spectral ----------------
    with tc.tile_pool(name="spec1", bufs=1) as sp1:
        T1 = sp1.tile([64, OC, 256], F32, tag="T1")              # (rmRI, c, w)
        T1v = T1.rearrange("p c w -> p (c w)")
        T1T0 = sp1.tile([128, 2, OC, 32], FP16, tag="T1T0")
        T1T1 = sp1.tile([128, 2, OC, 32], FP16, tag="T1T1")
        T1T = [T1T0, T1T1]

        with tc.tile_pool(name="xhp", bufs=2) as xhp, \
             tc.tile_pool(name="ps_t1", bufs=1, space="PSUM") as ps_t1, \
             tc.tile_pool(name="ps_tr", bufs=2, space="PSUM") as ps_tr:
            xh = []
            for ht in range(2):
                t = xhp.tile([128, OC, 256], FP16, tag="xh")
                nc.scalar.dma_start(
                    out=t, in_=P["xhh"][ht * 128:(ht + 1) * 128])
                xh.append(t)
            for reg in range(4):
                pt = ps_t1.tile([64, 2048], F32, tag="t1p")
                for n in range(4):
                    col = reg * 2048 + n * 512
                    for ht in range(2):
                        nc.tensor.matmul(
                            out=pt[:, n * 512:(n + 1) * 512],
                            lhsT=fhT_s[:, ht, :],
                            rhs=xh[ht].rearrange("p c w -> p (c w)")[:, col:col + 512],
                            start=(ht == 0), stop=(ht == 1))
                if reg % 2 == 0:
                    nc.vector.tensor_copy(out=T1v[:, reg * 2048:(reg + 1) * 2048],
                                          in_=pt)
                else:
                    nc.scalar.copy(out=T1v[:, reg * 2048:(reg + 1) * 2048], in_=pt)

            for blk in range(8):
                pt = ps_tr.tile([128, 512], F32, tag="trp")
                for i in range(8):
                    k = blk * 8 + i
                    cch, wh = k // 2, k % 2
                    nc.tensor.transpose(
                        out=pt[:, i * 64:(i + 1) * 64],
                        in_=T1[:, cch, wh * 128:(wh + 1) * 128],
                        identity=ident_s[0:64, 0:64])
                ptv = pt.rearrange("p (i w a b) -> p w a i b", i=4, w=2, a=2)
                for wh in range(2):
                    nc.scalar.copy(
                        out=T1T[wh][:, :, blk * 4:(blk + 1) * 4, :],
                        in_=ptv[:, wh])

        with tc.tile_pool(name="ps_cp", bufs=1, space="PSUM") as ps_cp, \
             tc.tile_pool(name="ps_zh", bufs=2, space="PSUM") as ps_zh:
            cp = ps_cp.tile([64, 16, 2, 32], F32, tag="cp")
            for pr in range(16):
                dst = cp[:, pr, :, :].rearrange("p a b -> p (a b)")
                for wh in range(2):
                    nc.tensor.matmul(out=dst,
                                     lhsT=T1T[wh][:, 0, 2 * pr:2 * pr + 2, :],
                                     rhs=fwA_s[:, wh, :],
                                     start=(wh == 0), stop=False)
                for wh in range(2):
                    nc.tensor.matmul(out=dst,
                                     lhsT=T1T[wh][:, 1, 2 * pr:2 * pr + 2, :],
                                     rhs=fwB_s[:, wh, :],
                                     start=False, stop=(wh == 1))
            tmpA = sp1.tile([32, 16, 32], F32, tag="mtmpA")
            tmpB = sp1.tile([32, 16, 32], F32, tag="mtmpB")
            for par in range(2):
                crs = cp[32 * par:32 * par + 32, :, 0, :]
                cis = cp[32 * par:32 * par + 32, :, 1, :]
                mr = mcR_s[:, par, :, :]
                mi = mcI_s[:, par, :, :]
                nc.vector.tensor_tensor(out=tmpA, in0=crs, in1=mr, op=AX.mult)
                nc.vector.tensor_tensor(out=tmpB, in0=cis, in1=mi, op=AX.mult)
                nc.vector.tensor_tensor(out=Qst[0:32, :, par, :], in0=tmpA, in1=tmpB,
                                        op=AX.subtract)
                nc.vector.tensor_tensor(out=tmpA, in0=cis, in1=mr, op=AX.mult)
                nc.vector.tensor_tensor(out=tmpB, in0=crs, in1=mi, op=AX.mult)
                nc.vector.tensor_tensor(out=Qst[32:64, :, par, :], in0=tmpA, in1=tmpB,
                                        op=AX.add)

            for grp in range(8):
                zp = ps_zh.tile([64, 4, 256], F32, tag="zhp")
                for i in range(4):
                    cch = grp * 4 + i
                    lhs = Qst_c[:, cch, :]
                    nc.tensor.matmul(out=zp[0:32, i, :], lhsT=lhs, rhs=ghR2_s,
                                     start=True, stop=True)
                    nc.tensor.matmul(out=zp[32:64, i, :], lhsT=lhs, rhs=ghI2_s,
                                     start=True, stop=True, tile_position=(0, 32))
                nc.scalar.copy(
                    out=Zh2e[0:64, :, grp * 4:(grp + 1) * 4, :],
                    in_=zp.rearrange("p c (q l) -> p q c l", q=4))

    # ---------------- S3 + S4: main loop ----------------
    with tc.tile_pool(name="xqp", bufs=3) as xqp, \
         tc.tile_pool(name="h1sp", bufs=2) as h1sp, \
         tc.tile_pool(name="chain", bufs=2) as chain, \
         tc.tile_pool(name="sqjp", bufs=2) as sqjp, \
         tc.tile_pool(name="ps_zp", bufs=3, space="PSUM") as ps_zp, \
         tc.tile_pool(name="ps_h1", bufs=1, space="PSUM") as ps_h1:
        for fc in range(NFC):
            f0 = fc * FC
            base0 = BB + RS * (1 + 8 * fc)
            ZPa = ps_zp.tile([128, 1024], F32, tag="ZP")
            ZPb = ps_zp.tile([128, 1024], F32, tag="ZP")
            ZPh = [ZPa, ZPb]
            xqp2 = []
            for qp in range(2):
                xq = xqp.tile([128, FC], FP16, tag="xq")
                for j in range(2):
                    q = qp * 2 + j
                    nc.gpsimd.dma_start(out=xq[64 * j:64 * j + 64, :],
                                        in_=xbh_f[:, q * FQ + f0:q * FQ + f0 + FC])
                xqp2.append(xq)
            xqs = [xqp2[q // 2][64 * (q % 2):64 * (q % 2) + 64, :] for q in range(NQ)]
            h1ss = []
            for q in range(NQ):
                h1s = h1sp.tile([128, FC], FP16, tag="h1s")
                j = q % 2
                for s in range(0, FC, 1024):
                    hp = ps_h1.tile([128, 1024], F32, tag="h1p")
                    for s2 in range(0, 1024, 512):
                        nc.tensor.matmul(
                            out=hp[:, s2:s2 + 512],
                            lhsT=wm1T2_s[64 * j:64 * j + 64, :],
                            rhs=xqs[q][:, s + s2:s + s2 + 512],
                            start=True, stop=True, tile_position=(64 * j, 0))
                    nc.scalar.activation(out=h1s[:, s:s + 1024], in_=hp,
                                         func=AF.Gelu, bias=bm1_s, scale=1.0)
                h1ss.append(h1s)
            # local (start=True)
            for qp in range(2):
                tp = (0, 64 * qp) if qp > 0 else None
                for s in range(0, FC, 512):
                    zsl = ZPh[s // 1024][64 * qp:64 * qp + 64, s % 1024:s % 1024 + 512]
                    nc.tensor.matmul(out=zsl, lhsT=wlocT2_s,
                                     rhs=xqp2[qp][:, s:s + 512],
                                     start=True, stop=False, tile_position=tp,
                                     skip_group_check=True)
            # PE dw taps
            for t, (dy, dx) in enumerate(PE_TAPS):
                d = dy * RS + dx
                for m in range(4):
                    rst = base0 + 2 * RS * m + 2 + d
                    rhs = xbp[:, rst:rst + 2 * RS].rearrange(
                        "p (r z) -> p r z", r=2)[:, :, 0:256]
                    zsl = ZPh[m // 2][:, (m % 2) * 512:(m % 2) * 512 + 512]
                    nc.tensor.matmul(out=zsl, lhsT=kdiag_s[:, t, :], rhs=rhs,
                                     start=False, stop=False, skip_group_check=True)
            # W-inverse y (+ bconst via 65th row)
            for i in range(NFC):
                h0 = fc * 8 + i
                lhs = Zh2e[:, :, :, h0]
                zsl = ZPh[i // 4][:, (i % 4) * 256:(i % 4) * 256 + 256]
                nc.tensor.matmul(out=zsl, lhsT=lhs, rhs=gw2e_s,
                                 start=False, stop=False, skip_group_check=True)
            # mlp2 (stop)
            for q in range(NQ):
                tp = (0, 32 * q) if q > 0 else None
                for s in range(0, FC, 512):
                    zsl = ZPh[s // 1024][32 * q:32 * q + 32, s % 1024:s % 1024 + 512]
                    nc.tensor.matmul(out=zsl, lhsT=wm2T_s, rhs=h1ss[q][:, s:s + 512],
                                     start=False, stop=True, tile_position=tp,
                                     skip_group_check=True)

            # --- DVE dw taps (the 2 horizontal ones) ---
            accD = chain.tile([128, 8 * RS], FP16, tag="accD")
            for t, (dy, dx) in enumerate(DVE_TAPS):
                d = dy * RS + dx
                xs = xbp[:, base0 + d:base0 + d + 8 * RS]
                nc.vector.scalar_tensor_tensor(out=accD, in0=xs,
                                               scalar=ktaps_s[:, t:t + 1],
                                               in1=xs if t == 0 else accD,
                                               op0=AX.mult,
                                               op1=AX.bypass if t == 0 else AX.add)
            # --- merge: zbuf = ZP + accD (fp16), accum -> szc ---
            accDv = accD.rearrange("p (r z) -> p r z", r=8)
            for hlf in range(2):
                nc.vector.scalar_tensor_tensor(
                    out=zbuf[:, f0 + hlf * 1024:f0 + hlf * 1024 + 1024],
                    in0=accDv[:, 4 * hlf:4 * hlf + 4, 2:258],
                    scalar=1.0,
                    in1=ZPh[hlf],
                    op0=AX.mult, op1=AX.add,
                    accum_out=szc[:, 2 * fc + hlf:2 * fc + hlf + 1])
            # --- sum(z^2) on ACT ---
            sqj = sqjp.tile([128, FC], FP16, tag="sqj")
            nc.scalar.activation(out=sqj, in_=zbuf[:, f0:f0 + FC], func=AF.Square,
                                 accum_out=sqc[:, fc:fc + 1])

    # ---------------- S5: stats ----------------
    st = ctx.enter_context(tc.tile_pool(name="stats", bufs=1))
    with tc.tile_pool(name="ps_st", bufs=1, space="PSUM") as ps_st:
        sums = st.tile([128, 2], F32, tag="sums")
        nc.vector.tensor_reduce(out=sums[:, 0:1], in_=szc,
                                axis=mybir.AxisListType.X, op=AX.add)
        nc.vector.tensor_reduce(out=sums[:, 1:2], in_=sqc,
                                axis=mybir.AxisListType.X, op=AX.add)
        sp = ps_st.tile([32, 2], F32, tag="sp")
        nc.tensor.matmul(out=sp, lhsT=qones_s, rhs=sums, start=True, stop=True)
        mu = st.tile([32, 1], F32, tag="mu")
        negmu = st.tile([32, 1], F32, tag="negmu")
        ex2 = st.tile([32, 1], F32, tag="ex2")
        var = st.tile([32, 1], F32, tag="var")
        s12 = st.tile([32, 2], F32, tag="s12")
        inv_n = 1.0 / float(HW)
        nc.vector.tensor_scalar(out=mu, in0=sp[:, 0:1], scalar1=inv_n,
                                scalar2=None, op0=AX.mult)
        nc.vector.tensor_scalar(out=negmu, in0=sp[:, 0:1], scalar1=-inv_n,
                                scalar2=None, op0=AX.mult)
        nc.vector.tensor_scalar(out=ex2, in0=sp[:, 1:2], scalar1=inv_n,
                                scalar2=None, op0=AX.mult)
        nc.vector.scalar_tensor_tensor(out=var, in0=mu, scalar=negmu, in1=ex2,
                                       op0=AX.mult, op1=AX.add)
        epst = st.tile([32, 1], F32, tag="epst")
        nc.vector.memset(epst, 1e-5)
        nc.scalar.activation(out=var, in_=var, func=AF.Sqrt, bias=epst, scale=1.0)
        nc.vector.reciprocal(out=var, in_=var)
        nc.vector.tensor_tensor(out=s12[:, 0:1], in0=var, in1=gam_s, op=AX.mult)
        nc.vector.tensor_scalar(out=negmu, in0=mu, scalar1=-1.0,
                                scalar2=None, op0=AX.mult)
        nc.vector.scalar_tensor_tensor(out=s12[:, 1:2], in0=s12[:, 0:1],
                                       scalar=negmu, in1=bet_s,
                                       op0=AX.mult, op1=AX.add)
        spb = ps_st.tile([128, 2], F32, tag="spb")
        nc.tensor.matmul(out=spb, lhsT=qonesT_s, rhs=s12, start=True, stop=True)
        s12s = st.tile([128, 2], F32, tag="s12s")
        nc.vector.tensor_copy(out=s12s, in_=spb)

    # ---------------- S6: fc pairs, gelu + residual, split queues ----------
    with tc.tile_pool(name="sw2", bufs=2) as sw2:
        for fp in range(4):
            f0 = fp * 2 * FC
            base0 = BB + RS * (1 + 16 * fp)
            g = sw2.tile([128, 2 * FC], FP16, tag="g")
            nc.scalar.activation(out=g, in_=zbuf[:, f0:f0 + 2 * FC], func=AF.Gelu,
                                 bias=s12s[:, 1:2], scale=s12s[:, 0:1])
            xwin = xbp[:, base0:base0 + 16 * RS].rearrange(
                "p (r z) -> p r z", r=16)[:, :, 2:258]
            gv = g.rearrange("p (r z) -> p r z", r=16)
            ob = sw2.tile([128, 2 * FC], F32, tag="ob")
            obv = ob.rearrange("p (r z) -> p r z", r=16)
            if fp == 3:
                nc.gpsimd.tensor_tensor(out=obv, in0=gv, in1=xwin, op=AX.add)
            else:
                nc.vector.tensor_tensor(out=obv, in0=gv, in1=xwin, op=AX.add)
            for q in range(NQ):
                eng = nc.sync if q % 2 == 0 else nc.scalar
                eng.dma_start(out=out_f[:, q * FQ + f0:q * FQ + f0 + 2 * FC],
                              in_=ob[32 * q:32 * q + 32, :])


_PROGRAM = None


def kernel(**inputs):
    global _PROGRAM
    in_maps = _per_core_inputs(inputs)
    if _PROGRAM is None:
        _PROGRAM = _build_program()
    res = run_bass_kernel_spmd(_PROGRAM, in_maps, list(range(N_CORES)))
    x = np.asarray(inputs["x"], np.float32)
    out = np.empty_like(x)
    for core in range(N_CORES):
        b, half = core // 2, core % 2
        out[b, half * 32:half * 32 + 32] = res.results[core]["outp"]
    return out


# revision 12
# speedup vs baseline: 1.0721x; 1.0721x over previous
"""Trainium2 Bass kernel for nn_CFTAOBlock2D (v3).

Sharding: pure data-parallel over (batch b, channel-half) -> 8 cores.
Each core gets its batch image with channels PERMUTED so its own 32 output
channels are always [0:32] (SPMD: one program, per-core data differs).

v3 vs v2 (281us):
  - host ships x in THREE layouts (c-major fp16, h-major fp16, and the
    padded own-channel (q,c)-layout) -> all loads are contiguous, no
    on-device memsets, tiny DMA descriptor counts.
  - consts packed into 5 params (was 21 DMAs).
  - 6 of 8 dw taps on PE (diag-matmul accumulate into z PSUM); the 2
    horizontal taps stay on DVE (stt is always 1x; fewer is better).
  - bconst folded into the y W-inverse matmul via a 65th contraction row.
  - S6 processed in fc-pairs: 1 gelu [128,4096], adds split DVE/gpsimd,
    out-DMA issue split across sync+scalar queues.
"""
from contextlib import ExitStack

import numpy as np

import concourse.bass as bass
import concourse.bacc as bacc
import concourse.tile as tile
from concourse import mybir
from concourse.bass_utils import run_bass_kernel_spmd

F32 = mybir.dt.float32
FP16 = mybir.dt.float16
AX = mybir.AluOpType
AF = mybir.ActivationFunctionType

B, C, H, W = 4, 64, 256, 256
M1, M2 = 32, 32
HALF_M = 16
LSEG, RADIAL_K = 4, 4
LOCAL_SCALE, SPATIAL_SCALE, SPEC_SCALE = 0.3, 0.15, 1.0
HW = H * W
NQ, QH = 4, 64
FQ = QH * W           # 16384
NFC = 8
FC = FQ // NFC        # 2048
OC = 32
N_CORES = 8

RS = W + 2            # 258 padded row stride
BB = 4
NROW = QH + 2         # 66 rows (2 halo)
XBLEN = 17040         # BB + NROW*RS = 17032, padded to mult of 8

PE_TAPS = [(-1, -1), (-1, 0), (-1, 1), (1, -1), (1, 0), (1, 1)]
DVE_TAPS = [(0, -1), (0, 1)]


# ---------------------------------------------------------------- host math
def _softplus(x):
    x = np.asarray(x, np.float64)
    return np.log1p(np.exp(-np.abs(x))) + np.maximum(x, 0.0)


def _softmax(x):
    e = np.exp(np.asarray(x, np.float64) - np.max(x))
    return e / e.sum()


def _modal_multiplier(f):
    gh = _softmax(f["seg_h_h"]) * LSEG
    gw = _softmax(f["seg_h_w"]) * LSEG
    seg_r = (np.arange(M1) * LSEG) // M1
    seg_c = (np.arange(M2) * LSEG) // M2
    seg_gain = gh[seg_r][:, None] * gw[seg_c][None, :]

    ky = np.linspace(0.0, 1.0, M1)
    kx = np.linspace(0.0, 1.0, M2)
    Ky, Kx = ky[:, None], kx[None, :]
    r2 = Ky * Ky + Kx * Kx
    r = np.sqrt(r2 + 1e-12)
    nu0 = _softplus(f["nu_log"])
    alpha0 = _softplus(f["alpha_log"])
    c_amp = _softplus(f["c_log"])
    amp_base = np.exp(-nu0 * r2) + c_amp / (1.0 + alpha0 * r2 + 1e-6)
    w0 = (r <= 0.33).astype(np.float64)
    w2b = (r >= 0.66).astype(np.float64)
    w1 = np.maximum(1.0 - w0 - w2b, 0.0)
    g = _softplus(f["band_gain"])
    amp_base = amp_base * ((1.0 + g[0]) * w0 + (1.0 + g[1]) * w1 + (1.0 + g[2]) * w2b)
    phi_base = np.float64(f["omega_y"]) * Ky + np.float64(f["omega_x"]) * Kx

    B_rad = np.stack([r**k for k in range(RADIAL_K)], axis=0)
    amp_delta = np.einsum("ck,khw->chw", _softplus(f["amp_coef"]), B_rad)
    phase_delta = np.einsum(
        "ck,khw->chw", np.asarray(f["phase_coef"], np.float64), B_rad)
    amp_full = amp_base[None] * (1.0 + np.maximum(amp_delta, 0.0))
    phi_full = phi_base[None] + phase_delta
    kernel = (np.cos(phi_full) + 1j * np.sin(phi_full)) * amp_full
    fk = (np.asarray(f["free_kernel_re"], np.float64)
          + 1j * np.asarray(f["free_kernel_im"], np.float64))
    return seg_gain[None] * kernel * SPEC_SCALE * (1.0 + np.float64(f["free_eps"]) * fk)


def _dft_mats():
    hh = np.arange(H)
    fr = np.concatenate([np.arange(HALF_M), np.arange(H - (M1 - HALF_M), H)])
    ang_h = 2.0 * np.pi * np.outer(hh, fr) / H
    fhT = np.concatenate([np.cos(ang_h), -np.sin(ang_h)], axis=1)  # (256, 64)

    ww = np.arange(W)
    mm = np.arange(M2)
    ang_w = 2.0 * np.pi * np.outer(ww, mm) / W
    FwR, FwI = np.cos(ang_w), -np.sin(ang_w)
    fwA = np.concatenate([FwR, FwI], axis=1)            # (256, 64)
    fwB = np.concatenate([-FwI, FwR], axis=1)           # (256, 64)

    GhR = np.cos(ang_h).T / H
    GhI = np.sin(ang_h).T / H
    ghR2 = np.concatenate([GhR, -GhI], axis=0)          # (64, 256)
    ghI2 = np.concatenate([GhI, GhR], axis=0)           # (64, 256)
    cm = np.full(M2, 2.0)
    cm[0] = 1.0
    GwR = (cm[:, None] * np.cos(ang_w.T)) / W
    GwI = (cm[:, None] * np.sin(ang_w.T)) / W
    gw2 = np.concatenate([GwR, -GwI], axis=0)           # (64, 256)
    return fhT, fwA, fwB, ghR2, ghI2, gw2


def _per_core_inputs(inputs):
    f = {k: np.asarray(v) for k, v in inputs.items()}
    x = np.asarray(f["x"], np.float32)
    Mc = _modal_multiplier(f)
    fhT, fwA, fwB, ghR2, ghI2, gw2 = _dft_mats()

    kd = (SPATIAL_SCALE * np.asarray(f["w_dw3"], np.float64)[:, 0])  # (64, 3, 3)
    w_local = np.asarray(f["w_local"], np.float64)
    w_mlp1 = np.asarray(f["w_mlp1"], np.float64)
    w_mlp2 = np.asarray(f["w_mlp2"], np.float64)
    b_local = np.asarray(f["b_local"], np.float64)
    b_dw3 = np.asarray(f["b_dw3"], np.float64)
    b_mlp1 = np.asarray(f["b_mlp1"], np.float64)
    b_mlp2 = np.asarray(f["b_mlp2"], np.float64)
    gamma = np.asarray(f["gamma"], np.float64)
    beta = np.asarray(f["beta"], np.float64)

    qones = np.zeros((128, 32), np.float32)
    for p in range(128):
        qones[p, p % 32] = 1.0
    qonesT = np.ascontiguousarray(qones.T)
    ident = np.eye(128, dtype=np.float32)

    in_maps = []
    for core in range(N_CORES):
        b, half = core // 2, core % 2
        perm = np.concatenate([np.arange(half * 32, half * 32 + 32),
                               np.arange((1 - half) * 32, (1 - half) * 32 + 32)])
        xbv16 = x[b][perm].astype(np.float16)           # (64, 256, 256)
        oc = perm[:OC]

        # h-major copy for the spectral H-DFT
        xhh = np.ascontiguousarray(xbv16.transpose(1, 0, 2))  # (256, 64, 256)[h,c,w]
        xhh = xhh[:, :OC, :]                                  # own channels only

        # padded own-channel (q,c) layout with halos, pads pre-zeroed
        xbp = np.zeros((128, XBLEN), np.float16)
        xo = xbv16[:OC].reshape(OC, H, W)
        for q in range(NQ):
            r0 = 64 * q - 1
            for s in range(NROW):
                r = r0 + s
                if r < 0 or r >= H:
                    continue
                xbp[32 * q:32 * q + 32, BB + s * RS + 2:BB + s * RS + 2 + W] = xo[:, r]

        mcR = np.empty((32, 2, 16, 32), np.float32)
        mcI = np.empty((32, 2, 16, 32), np.float32)
        for ci in range(OC):
            par, pair = ci % 2, ci // 2
            mcR[:, par, pair, :] = Mc[oc[ci]].real.astype(np.float32)
            mcI[:, par, pair, :] = Mc[oc[ci]].imag.astype(np.float32)

        wlocT = (LOCAL_SCALE * w_local[oc][:, perm].T)
        for ci in range(OC):
            wlocT[ci, ci] += kd[oc[ci], 1, 1]
        wlocT2 = np.zeros((128, 64), np.float64)
        wlocT2[0:64, 0:32] = wlocT
        wlocT2[64:128, 32:64] = wlocT
        wm1T = w_mlp1[:, perm].T
        wm1T2 = np.concatenate([wm1T, wm1T], axis=0)    # (128, 128)
        wm2T = w_mlp2[oc].T                             # (128, 32)

        ktaps = np.zeros((128, len(DVE_TAPS)), np.float32)
        for t, (dy, dx) in enumerate(DVE_TAPS):
            ktaps[:, t] = np.tile(kd[oc, dy + 1, dx + 1], NQ)
        kdiag = np.zeros((128, len(PE_TAPS), 128), np.float32)
        for t, (dy, dx) in enumerate(PE_TAPS):
            kv = np.tile(kd[oc, dy + 1, dx + 1], NQ)
            kdiag[np.arange(128), t, np.arange(128)] = kv

        bconst = (LOCAL_SCALE * b_local[oc] + SPATIAL_SCALE * b_dw3[oc] + b_mlp2[oc])
        bconst128 = np.tile(bconst, NQ)                 # (128,) in (q,c) order
        # 65th-row trick: bcrow goes into Zh2e[64]; gw2e row 64 is ones
        bcrow = np.broadcast_to(
            bconst128.reshape(1, NQ, 32, 1), (1, NQ, 32, 64)).astype(np.float16)

        # ---- packed consts ----
        ckA = np.concatenate([                                   # [128, *] fp16
            fhT.reshape(2, 128, 64).transpose(1, 0, 2).reshape(128, 128),
            fwA.reshape(2, 128, 64).transpose(1, 0, 2).reshape(128, 128),
            fwB.reshape(2, 128, 64).transpose(1, 0, 2).reshape(128, 128),
            wm1T2, wlocT2, wm2T,
            kdiag.reshape(128, len(PE_TAPS) * 128),
        ], axis=1).astype(np.float16)
        ckB = np.concatenate([ident, qones, ktaps,
                              b_mlp1.reshape(128, 1)], axis=1).astype(np.float32)
        gw2e = np.concatenate([gw2, np.ones((1, 256))], axis=0)  # (65, 256)
        ckC = np.zeros((65, 768), np.float16)
        ckC[0:64, 0:256] = ghR2.astype(np.float16)
        ckC[0:64, 256:512] = ghI2.astype(np.float16)
        ckC[:, 512:768] = gw2e.astype(np.float16)
        ckD = np.concatenate([
            qonesT, gamma[oc].reshape(32, 1), beta[oc].reshape(32, 1),
            mcR.reshape(32, 1024), mcI.reshape(32, 1024)], axis=1).astype(np.float32)

        in_maps.append({
            "xbh": np.ascontiguousarray(xbv16),
            "xhh": np.ascontiguousarray(xhh),
            "xbp": xbp,
            "bcrow": np.ascontiguousarray(bcrow),
            "ckA": np.ascontiguousarray(ckA),
            "ckB": np.ascontiguousarray(ckB),
            "ckC": np.ascontiguousarray(ckC),
            "ckD": np.ascontiguousarray(ckD),
        })
    return in_maps


CKA_W = 128 + 128 + 128 + 128 + 64 + 32 + len(PE_TAPS) * 128
CKB_W = 128 + 32 + len(DVE_TAPS) + 1
CKD_W = 128 + 1 + 1 + 1024 + 1024


# ---------------------------------------------------------------- device code
def _build_program():
    nc = bacc.Bacc(None, target_bir_lowering=False, debug=False)
    P = {}

    def di(name, shape, dtype=F32):
        P[name] = nc.declare_dram_parameter(name, list(shape), dtype, isOutput=False)

    di("xbh", (C, H, W), FP16)
    di("xhh", (H, OC, W), FP16)
    di("xbp", (128, XBLEN), FP16)
    di("bcrow", (1, NQ, 32, 64), FP16)
    di("ckA", (128, CKA_W), FP16)
    di("ckB", (128, CKB_W), F32)
    di("ckC", (65, 768), FP16)
    di("ckD", (32, CKD_W), F32)
    outp = nc.declare_dram_parameter("outp", [OC, H, W], F32, isOutput=True)

    with tile.TileContext(nc) as tc, ExitStack() as ctx:
        _body(ctx, tc, P, outp)
    nc.finalize()
    return nc


def _body(ctx, tc, P, outp):
    nc = tc.nc
    xbh = P["xbh"]
    xbh_f = xbh.rearrange("c h w -> c (h w)")                    # (64, 65536) fp16
    out_f = outp.rearrange("c h w -> c (h w)")                   # (32, 65536) f32

    consts = ctx.enter_context(tc.tile_pool(name="consts", bufs=1))
    ckA = consts.tile([128, CKA_W], FP16, tag="ckA")
    nc.sync.dma_start(out=ckA, in_=P["ckA"][:])
    ckB = consts.tile([128, CKB_W], F32, tag="ckB")
    nc.sync.dma_start(out=ckB, in_=P["ckB"][:])
    ckC = consts.tile([65, 768], FP16, tag="ckC")
    nc.sync.dma_start(out=ckC, in_=P["ckC"][:])
    ckD = consts.tile([32, CKD_W], F32, tag="ckD")
    nc.sync.dma_start(out=ckD, in_=P["ckD"][:])

    o = 0
    fhT_s = ckA[:, o:o + 128].rearrange("p (t m) -> p t m", t=2); o += 128
    fwA_s = ckA[:, o:o + 128].rearrange("p (t m) -> p t m", t=2); o += 128
    fwB_s = ckA[:, o:o + 128].rearrange("p (t m) -> p t m", t=2); o += 128
    wm1T2_s = ckA[:, o:o + 128]; o += 128
    wlocT2_s = ckA[:, o:o + 64]; o += 64
    wm2T_s = ckA[:, o:o + 32]; o += 32
    kdiag_s = ckA[:, o:o + len(PE_TAPS) * 128].rearrange(
        "p (t m) -> p t m", t=len(PE_TAPS)); o += len(PE_TAPS) * 128

    o = 0
    ident_s = ckB[:, o:o + 128]; o += 128
    qones_s = ckB[:, o:o + 32]; o += 32
    ktaps_s = ckB[:, o:o + len(DVE_TAPS)]; o += len(DVE_TAPS)
    bm1_s = ckB[:, o:o + 1]; o += 1

    ghR2_s = ckC[0:64, 0:256]
    ghI2_s = ckC[0:64, 256:512]
    gw2e_s = ckC[:, 512:768]                                     # (65, 256)

    o = 0
    qonesT_s = ckD[:, o:o + 128]; o += 128
    gam_s = ckD[:, o:o + 1]; o += 1
    bet_s = ckD[:, o:o + 1]; o += 1
    mcR_s = ckD[:, o:o + 1024].rearrange("p (a b c) -> p a b c", a=2, b=16); o += 1024
    mcI_s = ckD[:, o:o + 1024].rearrange("p (a b c) -> p a b c", a=2, b=16); o += 1024

    mid = ctx.enter_context(tc.tile_pool(name="mid", bufs=1))
    Qst = mid.tile([64, 16, 2, 32], FP16, tag="Qst")
    Qst_c = Qst.rearrange("p a b w -> p (a b) w")                # (64, 32, 32)
    # Zh2e: 65 x (q, c, hl); row 64 = bconst (bcrow), pairs with gw2e ones row
    Zh2e = mid.tile([65, 4, 32, 64], FP16, tag="Zh2e")
    nc.sync.dma_start(out=Zh2e[64:65], in_=P["bcrow"][:])

    main = ctx.enter_context(tc.tile_pool(name="main", bufs=1))
    xbp = main.tile([128, XBLEN], FP16, tag="xbp")
    nc.scalar.dma_start(out=xbp, in_=P["xbp"][:])  # after xhh on scalar
    zbuf = main.tile([128, FQ], FP16, tag="zbuf")
    szc = main.tile([128, 2 * NFC], F32, tag="szc")
    sqc = main.tile([128, NFC], F32, tag="sqc")

    # ---------------- S1 + S2: spectral ----------------
    with tc.tile_pool(name="spec1", bufs=1) as sp1:
        T1 = sp1.tile([64, OC, 256], F32, tag="T1")              # (rmRI, c, w)
        T1v = T1.rearrange("p c w -> p (c w)")
        T1T0 = sp1.tile([128, 2, OC, 32], FP16, tag="T1T0")
        T1T1 = sp1.tile([128, 2, OC, 32], FP16, tag="T1T1")
        T1T = [T1T0, T1T1]

        with tc.tile_pool(name="xhp", bufs=2) as xhp, \
             tc.tile_pool(name="ps_t1", bufs=1, space="PSUM") as ps_t1, \
             tc.tile_pool(name="ps_tr", bufs=2, space="PSUM") as ps_tr:
            xh = []
            for ht in range(2):
                t = xhp.tile([128, OC, 256], FP16, tag="xh")
                nc.scalar.dma_start(
                    out=t, in_=P["xhh"][ht * 128:(ht + 1) * 128])
                xh.append(t)
            for reg in range(4):
                pt = ps_t1.tile([64, 2048], F32, tag="t1p")
                for n in range(4):
                    col = reg * 2048 + n * 512
                    for ht in range(2):
                        nc.tensor.matmul(
                            out=pt[:, n * 512:(n + 1) * 512],
                            lhsT=fhT_s[:, ht, :],
                            rhs=xh[ht].rearrange("p c w -> p (c w)")[:, col:col + 512],
                            start=(ht == 0), stop=(ht == 1))
                nc.vector.tensor_copy(out=T1v[:, reg * 2048:(reg + 1) * 2048],
                                      in_=pt)

            for blk in range(8):
                pt = ps_tr.tile([128, 512], F32, tag="trp")
                for i in range(8):
                    k = blk * 8 + i
                    cch, wh = k // 2, k % 2
                    nc.tensor.transpose(
                        out=pt[:, i * 64:(i + 1) * 64],
                        in_=T1[:, cch, wh * 128:(wh + 1) * 128],
                        identity=ident_s[0:64, 0:64])
                ptv = pt.rearrange("p (i w a b) -> p w a i b", i=4, w=2, a=2)
                for wh in range(2):
                    nc.scalar.copy(
                        out=T1T[wh][:, :, blk * 4:(blk + 1) * 4, :],
                        in_=ptv[:, wh])

        with tc.tile_pool(name="ps_cp", bufs=1, space="PSUM") as ps_cp, \
             tc.tile_pool(name="ps_zh", bufs=2, space="PSUM") as ps_zh:
            cp = ps_cp.tile([64, 16, 2, 32], F32, tag="cp")
            for pr in range(16):
                dst = cp[:, pr, :, :].rearrange("p a b -> p (a b)")
                for wh in range(2):
                    nc.tensor.matmul(out=dst,
                                     lhsT=T1T[wh][:, 0, 2 * pr:2 * pr + 2, :],
                                     rhs=fwA_s[:, wh, :],
                                     start=(wh == 0), stop=False)
                for wh in range(2):
                    nc.tensor.matmul(out=dst,
                                     lhsT=T1T[wh][:, 1, 2 * pr:2 * pr + 2, :],
                                     rhs=fwB_s[:, wh, :],
                                     start=False, stop=(wh == 1))
            tmpA = sp1.tile([32, 16, 32], F32, tag="mtmpA")
            tmpB = sp1.tile([32, 16, 32], F32, tag="mtmpB")
            for par in range(2):
                crs = cp[32 * par:32 * par + 32, :, 0, :]
                cis = cp[32 * par:32 * par + 32, :, 1, :]
                mr = mcR_s[:, par, :, :]
                mi = mcI_s[:, par, :, :]
                nc.vector.tensor_tensor(out=tmpA, in0=crs, in1=mr, op=AX.mult)
                nc.vector.tensor_tensor(out=tmpB, in0=cis, in1=mi, op=AX.mult)
                nc.vector.tensor_tensor(out=Qst[0:32, :, par, :], in0=tmpA, in1=tmpB,
                                        op=AX.subtract)
                nc.vector.tensor_tensor(out=tmpA, in0=cis, in1=mr, op=AX.mult)
                nc.vector.tensor_tensor(out=tmpB, in0=crs, in1=mi, op=AX.mult)
                nc.vector.tensor_tensor(out=Qst[32:64, :, par, :], in0=tmpA, in1=tmpB,
                                        op=AX.add)

            for grp in range(8):
                zp = ps_zh.tile([64, 4, 256], F32, tag="zhp")
                for i in range(4):
                    cch = grp * 4 + i
                    lhs = Qst_c[:, cch, :]
                    nc.tensor.matmul(out=zp[0:32, i, :], lhsT=lhs, rhs=ghR2_s,
                                     start=True, stop=True)
                    nc.tensor.matmul(out=zp[32:64, i, :], lhsT=lhs, rhs=ghI2_s,
                                     start=True, stop=True, tile_position=(0, 32))
                nc.scalar.copy(
                    out=Zh2e[0:64, :, grp * 4:(grp + 1) * 4, :],
                    in_=zp.rearrange("p c (q l) -> p q c l", q=4))

    # ---------------- S3 + S4: main loop ----------------
    with tc.tile_pool(name="xqp", bufs=3) as xqp, \
         tc.tile_pool(name="h1sp", bufs=2) as h1sp, \
         tc.tile_pool(name="chain", bufs=2) as chain, \
         tc.tile_pool(name="sqjp", bufs=2) as sqjp, \
         tc.tile_pool(name="ps_zp", bufs=2, space="PSUM") as ps_zp, \
         tc.tile_pool(name="ps_h1", bufs=2, space="PSUM") as ps_h1:
        for fc in range(NFC):
            f0 = fc * FC
            base0 = BB + RS * (1 + 8 * fc)
            ZPa = ps_zp.tile([128, 1024], F32, tag="ZP")
            ZPb = ps_zp.tile([128, 1024], F32, tag="ZP")
            ZPh = [ZPa, ZPb]
            xqp2 = []
            for qp in range(2):
                xq = xqp.tile([128, FC], FP16, tag="xq")
                for j in range(2):
                    q = qp * 2 + j
                    nc.gpsimd.dma_start(out=xq[64 * j:64 * j + 64, :],
                                        in_=xbh_f[:, q * FQ + f0:q * FQ + f0 + FC])
                xqp2.append(xq)
            xqs = [xqp2[q // 2][64 * (q % 2):64 * (q % 2) + 64, :] for q in range(NQ)]
            h1ss = []
            for q in range(NQ):
                h1s = h1sp.tile([128, FC], FP16, tag="h1s")
                j = q % 2
                for s in range(0, FC, 1024):
                    hp = ps_h1.tile([128, 1024], F32, tag="h1p")
                    for s2 in range(0, 1024, 512):
                        nc.tensor.matmul(
                            out=hp[:, s2:s2 + 512],
                            lhsT=wm1T2_s[64 * j:64 * j + 64, :],
                            rhs=xqs[q][:, s + s2:s + s2 + 512],
                            start=True, stop=True, tile_position=(64 * j, 0))
                    nc.scalar.activation(out=h1s[:, s:s + 1024], in_=hp,
                                         func=AF.Gelu, bias=bm1_s, scale=1.0)
                h1ss.append(h1s)
            # local (start=True)
            for qp in range(2):
                tp = (0, 64 * qp) if qp > 0 else None
                for s in range(0, FC, 512):
                    zsl = ZPh[s // 1024][64 * qp:64 * qp + 64, s % 1024:s % 1024 + 512]
                    nc.tensor.matmul(out=zsl, lhsT=wlocT2_s,
                                     rhs=xqp2[qp][:, s:s + 512],
                                     start=True, stop=False, tile_position=tp,
                                     skip_group_check=True)
            # PE dw taps
            for t, (dy, dx) in enumerate(PE_TAPS):
                d = dy * RS + dx
                for m in range(4):
                    rst = base0 + 2 * RS * m + 2 + d
                    rhs = xbp[:, rst:rst + 2 * RS].rearrange(
                        "p (r z) -> p r z", r=2)[:, :, 0:256]
                    zsl = ZPh[m // 2][:, (m % 2) * 512:(m % 2) * 512 + 512]
                    nc.tensor.matmul(out=zsl, lhsT=kdiag_s[:, t, :], rhs=rhs,
                                     start=False, stop=False, skip_group_check=True)
            # W-inverse y (+ bconst via 65th row)
            for i in range(NFC):
                h0 = fc * 8 + i
                lhs = Zh2e[:, :, :, h0]
                zsl = ZPh[i // 4][:, (i % 4) * 256:(i % 4) * 256 + 256]
                nc.tensor.matmul(out=zsl, lhsT=lhs, rhs=gw2e_s,
                                 start=False, stop=False, skip_group_check=True)
            # mlp2 (stop)
            for q in range(NQ):
                tp = (0, 32 * q) if q > 0 else None
                for s in range(0, FC, 512):
                    zsl = ZPh[s // 1024][32 * q:32 * q + 32, s % 1024:s % 1024 + 512]
                    nc.tensor.matmul(out=zsl, lhsT=wm2T_s, rhs=h1ss[q][:, s:s + 512],
                                     start=False, stop=True, tile_position=tp,
                                     skip_group_check=True)

            # --- DVE dw taps (the 2 horizontal ones) ---
            accD = chain.tile([128, 8 * RS], FP16, tag="accD")
            for t, (dy, dx) in enumerate(DVE_TAPS):
                d = dy * RS + dx
                xs = xbp[:, base0 + d:base0 + d + 8 * RS]
                nc.vector.scalar_tensor_tensor(out=accD, in0=xs,
                                               scalar=ktaps_s[:, t:t + 1],
                                               in1=xs if t == 0 else accD,
                                               op0=AX.mult,
                                               op1=AX.bypass if t == 0 else AX.add)
            # --- merge: zbuf = ZP + accD (fp16), accum -> szc ---
            accDv = accD.rearrange("p (r z) -> p r z", r=8)
            for hlf in range(2):
                nc.vector.scalar_tensor_tensor(
                    out=zbuf[:, f0 + hlf * 1024:f0 + hlf * 1024 + 1024],
                    in0=accDv[:, 4 * hlf:4 * hlf + 4, 2:258],
                    scalar=1.0,
                    in1=ZPh[hlf],
                    op0=AX.mult, op1=AX.add,
                    accum_out=szc[:, 2 * fc + hlf:2 * fc + hlf + 1])
            # --- sum(z^2) on ACT ---
            sqj = sqjp.tile([128, FC], FP16, tag="sqj")
            nc.scalar.activation(out=sqj, in_=zbuf[:, f0:f0 + FC], func=AF.Square,
                                 accum_out=sqc[:, fc:fc + 1])

    # ---------------- S5: stats ----------------
    st = ctx.enter_context(tc.tile_pool(name="stats", bufs=1))
    with tc.tile_pool(name="ps_st", bufs=1, space="PSUM") as ps_st:
        sums = st.tile([128, 2], F32, tag="sums")
        nc.vector.tensor_reduce(out=sums[:, 0:1], in_=szc,
                                axis=mybir.AxisListType.X, op=AX.add)
        nc.vector.tensor_reduce(out=sums[:, 1:2], in_=sqc,
                                axis=mybir.AxisListType.X, op=AX.add)
        sp = ps_st.tile([32, 2], F32, tag="sp")
        nc.tensor.matmul(out=sp, lhsT=qones_s, rhs=sums, start=True, stop=True)
        mu = st.tile([32, 1], F32, tag="mu")
        negmu = st.tile([32, 1], F32, tag="negmu")
        ex2 = st.tile([32, 1], F32, tag="ex2")
        var = st.tile([32, 1], F32, tag="var")
        s12 = st.tile([32, 2], F32, tag="s12")
        inv_n = 1.0 / float(HW)
        nc.vector.tensor_scalar(out=mu, in0=sp[:, 0:1], scalar1=inv_n,
                                scalar2=None, op0=AX.mult)
        nc.vector.tensor_scalar(out=negmu, in0=sp[:, 0:1], scalar1=-inv_n,
                                scalar2=None, op0=AX.mult)
        nc.vector.tensor_scalar(out=ex2, in0=sp[:, 1:2], scalar1=inv_n,
                                scalar2=None, op0=AX.mult)
        nc.vector.scalar_tensor_tensor(out=var, in0=mu, scalar=negmu, in1=ex2,
                                       op0=AX.mult, op1=AX.add)
        epst = st.tile([32, 1], F32, tag="epst")
        nc.vector.memset(epst, 1e-5)
        nc.scalar.activation(out=var, in_=var, func=AF.Sqrt, bias=epst, scale=1.0)
        nc.vector.reciprocal(out=var, in_=var)
        nc.vector.tensor_tensor(out=s12[:, 0:1], in0=var, in1=gam_s, op=AX.mult)
        nc.vector.tensor_scalar(out=negmu, in0=mu, scalar1=-1.0,
                                scalar2=None, op0=AX.mult)
        nc.vector.scalar_tensor_tensor(out=s12[:, 1:2], in0=s12[:, 0:1],
                                       scalar=negmu, in1=bet_s,
                                       op0=AX.mult, op1=AX.add)
        spb = ps_st.tile([128, 2], F32, tag="spb")
        nc.tensor.matmul(out=spb, lhsT=qonesT_s, rhs=s12, start=True, stop=True)
        s12s = st.tile([128, 2], F32, tag="s12s")
        nc.vector.tensor_copy(out=s12s, in_=spb)

    # ---------------- S6: fc pairs, gelu + residual, split queues ----------
    with tc.tile_pool(name="sw2", bufs=3) as sw2:
        for fc in range(NFC):
            f0 = fc * FC
            base0 = BB + RS * (1 + 8 * fc)
            g = sw2.tile([128, FC], FP16, tag="g")
            nc.scalar.activation(out=g, in_=zbuf[:, f0:f0 + FC], func=AF.Gelu,
                                 bias=s12s[:, 1:2], scale=s12s[:, 0:1])
            xwin = xbp[:, base0:base0 + 8 * RS].rearrange(
                "p (r z) -> p r z", r=8)[:, :, 2:258]
            gv = g.rearrange("p (r z) -> p r z", r=8)
            ob = sw2.tile([128, FC], F32, tag="ob")
            obv = ob.rearrange("p (r z) -> p r z", r=8)
            nc.vector.tensor_tensor(out=obv, in0=gv, in1=xwin, op=AX.add)
            for q in range(NQ):
                eng = nc.sync if q % 2 == 0 else nc.gpsimd
                eng.dma_start(out=out_f[:, q * FQ + f0:q * FQ + f0 + FC],
                              in_=ob[32 * q:32 * q + 32, :])


_PROGRAM = None


def kernel(**inputs):
    global _PROGRAM
    in_maps = _per_core_inputs(inputs)
    if _PROGRAM is None:
        _PROGRAM = _build_program()
    res = run_bass_kernel_spmd(_PROGRAM, in_maps, list(range(N_CORES)))
    x = np.asarray(inputs["x"], np.float32)
    out = np.empty_like(x)
    for core in range(N_CORES):
        b, half = core // 2, core % 2
        out[b, half * 32:half * 32 + 32] = res.results[core]["outp"]
    return out


# revision 14
# speedup vs baseline: 1.2228x; 1.1405x over previous
"""Trainium2 Bass kernel for nn_CFTAOBlock2D (v3).

Sharding: pure data-parallel over (batch b, channel-half) -> 8 cores.
Each core gets its batch image with channels PERMUTED so its own 32 output
channels are always [0:32] (SPMD: one program, per-core data differs).

v3 vs v2 (281us):
  - host ships x in THREE layouts (c-major fp16, h-major fp16, and the
    padded own-channel (q,c)-layout) -> all loads are contiguous, no
    on-device memsets, tiny DMA descriptor counts.
  - consts packed into 5 params (was 21 DMAs).
  - 6 of 8 dw taps on PE (diag-matmul accumulate into z PSUM); the 2
    horizontal taps stay on DVE (stt is always 1x; fewer is better).
  - bconst folded into the y W-inverse matmul via a 65th contraction row.
  - S6 processed in fc-pairs: 1 gelu [128,4096], adds split DVE/gpsimd,
    out-DMA issue split across sync+scalar queues.
"""
from contextlib import ExitStack

import numpy as np

import concourse.bass as bass
import concourse.bacc as bacc
import concourse.tile as tile
from concourse import mybir
from concourse.bass_utils import run_bass_kernel_spmd

F32 = mybir.dt.float32
FP16 = mybir.dt.float16
AX = mybir.AluOpType
AF = mybir.ActivationFunctionType

B, C, H, W = 4, 64, 256, 256
M1, M2 = 32, 32
HALF_M = 16
LSEG, RADIAL_K = 4, 4
LOCAL_SCALE, SPATIAL_SCALE, SPEC_SCALE = 0.3, 0.15, 1.0
HW = H * W
NQ, QH = 4, 64
FQ = QH * W           # 16384
NFC = 8
FC = FQ // NFC        # 2048
OC = 32
N_CORES = 8

RS = W + 2            # 258 padded row stride
BB = 4
NROW = QH + 2         # 66 rows (2 halo)
XBLEN = 17040         # BB + NROW*RS = 17032, padded to mult of 8

PE_TAPS = [(-1, -1), (-1, 0), (-1, 1), (1, -1), (1, 0), (1, 1)]
DVE_TAPS = [(0, -1), (0, 1)]


# ---------------------------------------------------------------- host math
def _softplus(x):
    x = np.asarray(x, np.float64)
    return np.log1p(np.exp(-np.abs(x))) + np.maximum(x, 0.0)


def _softmax(x):
    e = np.exp(np.asarray(x, np.float64) - np.max(x))
    return e / e.sum()


def _modal_multiplier(f):
    gh = _softmax(f["seg_h_h"]) * LSEG
    gw = _softmax(f["seg_h_w"]) * LSEG
    seg_r = (np.arange(M1) * LSEG) // M1
    seg_c = (np.arange(M2) * LSEG) // M2
    seg_gain = gh[seg_r][:, None] * gw[seg_c][None, :]

    ky = np.linspace(0.0, 1.0, M1)
    kx = np.linspace(0.0, 1.0, M2)
    Ky, Kx = ky[:, None], kx[None, :]
    r2 = Ky * Ky + Kx * Kx
    r = np.sqrt(r2 + 1e-12)
    nu0 = _softplus(f["nu_log"])
    alpha0 = _softplus(f["alpha_log"])
    c_amp = _softplus(f["c_log"])
    amp_base = np.exp(-nu0 * r2) + c_amp / (1.0 + alpha0 * r2 + 1e-6)
    w0 = (r <= 0.33).astype(np.float64)
    w2b = (r >= 0.66).astype(np.float64)
    w1 = np.maximum(1.0 - w0 - w2b, 0.0)
    g = _softplus(f["band_gain"])
    amp_base = amp_base * ((1.0 + g[0]) * w0 + (1.0 + g[1]) * w1 + (1.0 + g[2]) * w2b)
    phi_base = np.float64(f["omega_y"]) * Ky + np.float64(f["omega_x"]) * Kx

    B_rad = np.stack([r**k for k in range(RADIAL_K)], axis=0)
    amp_delta = np.einsum("ck,khw->chw", _softplus(f["amp_coef"]), B_rad)
    phase_delta = np.einsum(
        "ck,khw->chw", np.asarray(f["phase_coef"], np.float64), B_rad)
    amp_full = amp_base[None] * (1.0 + np.maximum(amp_delta, 0.0))
    phi_full = phi_base[None] + phase_delta
    kernel = (np.cos(phi_full) + 1j * np.sin(phi_full)) * amp_full
    fk = (np.asarray(f["free_kernel_re"], np.float64)
          + 1j * np.asarray(f["free_kernel_im"], np.float64))
    return seg_gain[None] * kernel * SPEC_SCALE * (1.0 + np.float64(f["free_eps"]) * fk)


def _dft_mats():
    hh = np.arange(H)
    fr = np.concatenate([np.arange(HALF_M), np.arange(H - (M1 - HALF_M), H)])
    ang_h = 2.0 * np.pi * np.outer(hh, fr) / H
    fhT = np.concatenate([np.cos(ang_h), -np.sin(ang_h)], axis=1)  # (256, 64)

    ww = np.arange(W)
    mm = np.arange(M2)
    ang_w = 2.0 * np.pi * np.outer(ww, mm) / W
    FwR, FwI = np.cos(ang_w), -np.sin(ang_w)
    fwA = np.concatenate([FwR, FwI], axis=1)            # (256, 64)
    fwB = np.concatenate([-FwI, FwR], axis=1)           # (256, 64)

    GhR = np.cos(ang_h).T / H
    GhI = np.sin(ang_h).T / H
    ghR2 = np.concatenate([GhR, -GhI], axis=0)          # (64, 256)
    ghI2 = np.concatenate([GhI, GhR], axis=0)           # (64, 256)
    cm = np.full(M2, 2.0)
    cm[0] = 1.0
    GwR = (cm[:, None] * np.cos(ang_w.T)) / W
    GwI = (cm[:, None] * np.sin(ang_w.T)) / W
    gw2 = np.concatenate([GwR, -GwI], axis=0)           # (64, 256)
    return fhT, fwA, fwB, ghR2, ghI2, gw2


def _per_core_inputs(inputs):
    f = {k: np.asarray(v) for k, v in inputs.items()}
    x = np.asarray(f["x"], np.float32)
    Mc = _modal_multiplier(f)
    fhT, fwA, fwB, ghR2, ghI2, gw2 = _dft_mats()

    kd = (SPATIAL_SCALE * np.asarray(f["w_dw3"], np.float64)[:, 0])  # (64, 3, 3)
    w_local = np.asarray(f["w_local"], np.float64)
    w_mlp1 = np.asarray(f["w_mlp1"], np.float64)
    w_mlp2 = np.asarray(f["w_mlp2"], np.float64)
    b_local = np.asarray(f["b_local"], np.float64)
    b_dw3 = np.asarray(f["b_dw3"], np.float64)
    b_mlp1 = np.asarray(f["b_mlp1"], np.float64)
    b_mlp2 = np.asarray(f["b_mlp2"], np.float64)
    gamma = np.asarray(f["gamma"], np.float64)
    beta = np.asarray(f["beta"], np.float64)

    qones = np.zeros((128, 32), np.float32)
    for p in range(128):
        qones[p, p % 32] = 1.0
    qonesT = np.ascontiguousarray(qones.T)
    ident = np.eye(128, dtype=np.float32)

    in_maps = []
    for core in range(N_CORES):
        b, half = core // 2, core % 2
        perm = np.concatenate([np.arange(half * 32, half * 32 + 32),
                               np.arange((1 - half) * 32, (1 - half) * 32 + 32)])
        xbv16 = x[b][perm].astype(np.float16)           # (64, 256, 256)
        oc = perm[:OC]

        # h-major copy for the spectral H-DFT
        xhh = np.ascontiguousarray(xbv16.transpose(1, 0, 2))  # (256, 64, 256)[h,c,w]
        xhh = xhh[:, :OC, :]                                  # own channels only

        # padded own-channel (q,c) layout with halos, pads pre-zeroed
        xbp = np.zeros((128, XBLEN), np.float16)
        xo = xbv16[:OC].reshape(OC, H, W)
        for q in range(NQ):
            r0 = 64 * q - 1
            for s in range(NROW):
                r = r0 + s
                if r < 0 or r >= H:
                    continue
                xbp[32 * q:32 * q + 32, BB + s * RS + 2:BB + s * RS + 2 + W] = xo[:, r]

        mcR = np.empty((32, 2, 16, 32), np.float32)
        mcI = np.empty((32, 2, 16, 32), np.float32)
        for ci in range(OC):
            par, pair = ci % 2, ci // 2
            mcR[:, par, pair, :] = Mc[oc[ci]].real.astype(np.float32)
            mcI[:, par, pair, :] = Mc[oc[ci]].imag.astype(np.float32)

        wlocT = (LOCAL_SCALE * w_local[oc][:, perm].T)
        for ci in range(OC):
            wlocT[ci, ci] += kd[oc[ci], 1, 1]
        wlocT2 = np.zeros((128, 64), np.float64)
        wlocT2[0:64, 0:32] = wlocT
        wlocT2[64:128, 32:64] = wlocT
        wm1T = w_mlp1[:, perm].T
        wm1T2 = np.concatenate([wm1T, wm1T], axis=0)    # (128, 128)
        wm2T = w_mlp2[oc].T                             # (128, 32)

        ktaps = np.zeros((128, len(DVE_TAPS)), np.float32)
        for t, (dy, dx) in enumerate(DVE_TAPS):
            ktaps[:, t] = np.tile(kd[oc, dy + 1, dx + 1], NQ)
        kdiag = np.zeros((128, len(PE_TAPS), 128), np.float32)
        for t, (dy, dx) in enumerate(PE_TAPS):
            kv = np.tile(kd[oc, dy + 1, dx + 1], NQ)
            kdiag[np.arange(128), t, np.arange(128)] = kv

        bconst = (LOCAL_SCALE * b_local[oc] + SPATIAL_SCALE * b_dw3[oc] + b_mlp2[oc])
        bconst128 = np.tile(bconst, NQ)                 # (128,) in (q,c) order
        # 65th-row trick: bcrow goes into Zh2e[64]; gw2e row 64 is ones
        bcrow = np.broadcast_to(
            bconst128.reshape(1, NQ, 32, 1), (1, NQ, 32, 64)).astype(np.float16)

        # ---- packed consts ----
        ckA = np.concatenate([                                   # [128, *] fp16
            fhT.reshape(2, 128, 64).transpose(1, 0, 2).reshape(128, 128),
            fwA.reshape(2, 128, 64).transpose(1, 0, 2).reshape(128, 128),
            fwB.reshape(2, 128, 64).transpose(1, 0, 2).reshape(128, 128),
            wm1T2, wlocT2, wm2T,
            kdiag.reshape(128, len(PE_TAPS) * 128),
        ], axis=1).astype(np.float16)
        ckB = np.concatenate([ident, qones, ktaps,
                              b_mlp1.reshape(128, 1)], axis=1).astype(np.float32)
        gw2e = np.concatenate([gw2, np.ones((1, 256))], axis=0)  # (65, 256)
        ckC = np.zeros((65, 768), np.float16)
        ckC[0:64, 0:256] = ghR2.astype(np.float16)
        ckC[0:64, 256:512] = ghI2.astype(np.float16)
        ckC[:, 512:768] = gw2e.astype(np.float16)
        ckD = np.concatenate([
            qonesT, gamma[oc].reshape(32, 1), beta[oc].reshape(32, 1),
            mcR.reshape(32, 1024), mcI.reshape(32, 1024)], axis=1).astype(np.float32)

        in_maps.append({
            "xbh": np.ascontiguousarray(xbv16),
            "xhh": np.ascontiguousarray(xhh),
            "xbp": xbp,
            "bcrow": np.ascontiguousarray(bcrow),
            "ckA": np.ascontiguousarray(ckA),
            "ckB": np.ascontiguousarray(ckB),
            "ckC": np.ascontiguousarray(ckC),
            "ckD": np.ascontiguousarray(ckD),
        })
    return in_maps


CKA_W = 128 + 128 + 128 + 128 + 64 + 32 + len(PE_TAPS) * 128
CKB_W = 128 + 32 + len(DVE_TAPS) + 1
CKD_W = 128 + 1 + 1 + 1024 + 1024


# ---------------------------------------------------------------- device code
def _build_program():
    nc = bacc.Bacc(None, target_bir_lowering=False, debug=False)
    P = {}

    def di(name, shape, dtype=F32):
        P[name] = nc.declare_dram_parameter(name, list(shape), dtype, isOutput=False)

    di("xbh", (C, H, W), FP16)
    di("xhh", (H, OC, W), FP16)
    di("xbp", (128, XBLEN), FP16)
    di("bcrow", (1, NQ, 32, 64), FP16)
    di("ckA", (128, CKA_W), FP16)
    di("ckB", (128, CKB_W), F32)
    di("ckC", (65, 768), FP16)
    di("ckD", (32, CKD_W), F32)
    outp = nc.declare_dram_parameter("outp", [OC, H, W], F32, isOutput=True)

    with tile.TileContext(nc) as tc, ExitStack() as ctx:
        _body(ctx, tc, P, outp)
    nc.finalize()
    return nc


def _body(ctx, tc, P, outp):
    nc = tc.nc
    xbh = P["xbh"]
    xbh_f = xbh.rearrange("c h w -> c (h w)")                    # (64, 65536) fp16
    out_f = outp.rearrange("c h w -> c (h w)")                   # (32, 65536) f32

    consts = ctx.enter_context(tc.tile_pool(name="consts", bufs=1))
    ckA = consts.tile([128, CKA_W], FP16, tag="ckA")
    nc.sync.dma_start(out=ckA, in_=P["ckA"][:])
    ckB = consts.tile([128, CKB_W], F32, tag="ckB")
    nc.sync.dma_start(out=ckB, in_=P["ckB"][:])
    ckC = consts.tile([65, 768], FP16, tag="ckC")
    nc.sync.dma_start(out=ckC, in_=P["ckC"][:])
    ckD = consts.tile([32, CKD_W], F32, tag="ckD")
    nc.sync.dma_start(out=ckD, in_=P["ckD"][:])

    o = 0
    fhT_s = ckA[:, o:o + 128].rearrange("p (t m) -> p t m", t=2); o += 128
    fwA_s = ckA[:, o:o + 128].rearrange("p (t m) -> p t m", t=2); o += 128
    fwB_s = ckA[:, o:o + 128].rearrange("p (t m) -> p t m", t=2); o += 128
    wm1T2_s = ckA[:, o:o + 128]; o += 128
    wlocT2_s = ckA[:, o:o + 64]; o += 64
    wm2T_s = ckA[:, o:o + 32]; o += 32
    kdiag_s = ckA[:, o:o + len(PE_TAPS) * 128].rearrange(
        "p (t m) -> p t m", t=len(PE_TAPS)); o += len(PE_TAPS) * 128

    o = 0
    ident_s = ckB[:, o:o + 128]; o += 128
    qones_s = ckB[:, o:o + 32]; o += 32
    ktaps_s = ckB[:, o:o + len(DVE_TAPS)]; o += len(DVE_TAPS)
    bm1_s = ckB[:, o:o + 1]; o += 1

    ghR2_s = ckC[0:64, 0:256]
    ghI2_s = ckC[0:64, 256:512]
    gw2e_s = ckC[:, 512:768]                                     # (65, 256)

    o = 0
    qonesT_s = ckD[:, o:o + 128]; o += 128
    gam_s = ckD[:, o:o + 1]; o += 1
    bet_s = ckD[:, o:o + 1]; o += 1
    mcR_s = ckD[:, o:o + 1024].rearrange("p (a b c) -> p a b c", a=2, b=16); o += 1024
    mcI_s = ckD[:, o:o + 1024].rearrange("p (a b c) -> p a b c", a=2, b=16); o += 1024

    mid = ctx.enter_context(tc.tile_pool(name="mid", bufs=1))
    Qst = mid.tile([64, 16, 2, 32], FP16, tag="Qst")
    Qst_c = Qst.rearrange("p a b w -> p (a b) w")                # (64, 32, 32)
    # Zh2e: 65 x (q, c, hl); row 64 = bconst (bcrow), pairs with gw2e ones row
    Zh2e = mid.tile([65, 4, 32, 64], FP16, tag="Zh2e")
    nc.sync.dma_start(out=Zh2e[64:65], in_=P["bcrow"][:])

    main = ctx.enter_context(tc.tile_pool(name="main", bufs=1))
    xbp = main.tile([128, XBLEN], FP16, tag="xbp")
    nc.scalar.dma_start(out=xbp, in_=P["xbp"][:])  # after xhh on scalar
    zbuf = main.tile([128, FQ], FP16, tag="zbuf")
    szc = main.tile([128, 2 * NFC], F32, tag="szc")
    sqc = main.tile([128, NFC], F32, tag="sqc")

    # ---------------- S1 + S2: spectral ----------------
    with tc.tile_pool(name="spec1", bufs=1) as sp1:
        T1 = sp1.tile([64, OC, 256], F32, tag="T1")              # (rmRI, c, w)
        T1v = T1.rearrange("p c w -> p (c w)")
        T1T0 = sp1.tile([128, 2, OC, 32], FP16, tag="T1T0")
        T1T1 = sp1.tile([128, 2, OC, 32], FP16, tag="T1T1")
        T1T = [T1T0, T1T1]

        with tc.tile_pool(name="xhp", bufs=2) as xhp, \
             tc.tile_pool(name="ps_t1", bufs=2, space="PSUM") as ps_t1, \
             tc.tile_pool(name="ps_tr", bufs=2, space="PSUM") as ps_tr:
            xh = []
            for ht in range(2):
                t = xhp.tile([128, OC, 256], FP16, tag="xh")
                nc.scalar.dma_start(
                    out=t, in_=P["xhh"][ht * 128:(ht + 1) * 128])
                xh.append(t)
            for reg in range(8):
                pt = ps_t1.tile([64, 1024], F32, tag="t1p")
                for n in range(2):
                    col = reg * 1024 + n * 512
                    for ht in range(2):
                        nc.tensor.matmul(
                            out=pt[:, n * 512:(n + 1) * 512],
                            lhsT=fhT_s[:, ht, :],
                            rhs=xh[ht].rearrange("p c w -> p (c w)")[:, col:col + 512],
                            start=(ht == 0), stop=(ht == 1))
                if reg % 2 == 0:
                    nc.vector.tensor_copy(out=T1v[:, reg * 1024:(reg + 1) * 1024],
                                          in_=pt)
                else:
                    nc.scalar.copy(out=T1v[:, reg * 1024:(reg + 1) * 1024], in_=pt)

            for blk in range(8):
                pt = ps_tr.tile([128, 512], F32, tag="trp")
                for i in range(8):
                    k = blk * 8 + i
                    cch, wh = k // 2, k % 2
                    nc.tensor.transpose(
                        out=pt[:, i * 64:(i + 1) * 64],
                        in_=T1[:, cch, wh * 128:(wh + 1) * 128],
                        identity=ident_s[0:64, 0:64])
                ptv = pt.rearrange("p (i w a b) -> p w a i b", i=4, w=2, a=2)
                for wh in range(2):
                    nc.scalar.copy(
                        out=T1T[wh][:, :, blk * 4:(blk + 1) * 4, :],
                        in_=ptv[:, wh])

        with tc.tile_pool(name="ps_cp", bufs=1, space="PSUM") as ps_cp, \
             tc.tile_pool(name="ps_zh", bufs=2, space="PSUM") as ps_zh:
            cp = ps_cp.tile([64, 16, 2, 32], F32, tag="cp")
            for pr in range(16):
                dst = cp[:, pr, :, :].rearrange("p a b -> p (a b)")
                for wh in range(2):
                    nc.tensor.matmul(out=dst,
                                     lhsT=T1T[wh][:, 0, 2 * pr:2 * pr + 2, :],
                                     rhs=fwA_s[:, wh, :],
                                     start=(wh == 0), stop=False)
                for wh in range(2):
                    nc.tensor.matmul(out=dst,
                                     lhsT=T1T[wh][:, 1, 2 * pr:2 * pr + 2, :],
                                     rhs=fwB_s[:, wh, :],
                                     start=False, stop=(wh == 1))
            tmpA = sp1.tile([32, 16, 32], F32, tag="mtmpA")
            tmpB = sp1.tile([32, 16, 32], F32, tag="mtmpB")
            for par in range(2):
                crs = cp[32 * par:32 * par + 32, :, 0, :]
                cis = cp[32 * par:32 * par + 32, :, 1, :]
                mr = mcR_s[:, par, :, :]
                mi = mcI_s[:, par, :, :]
                nc.vector.tensor_tensor(out=tmpA, in0=crs, in1=mr, op=AX.mult)
                nc.vector.tensor_tensor(out=tmpB, in0=cis, in1=mi, op=AX.mult)
                nc.vector.tensor_tensor(out=Qst[0:32, :, par, :], in0=tmpA, in1=tmpB,
                                        op=AX.subtract)
                nc.vector.tensor_tensor(out=tmpA, in0=cis, in1=mr, op=AX.mult)
                nc.vector.tensor_tensor(out=tmpB, in0=crs, in1=mi, op=AX.mult)
                nc.vector.tensor_tensor(out=Qst[32:64, :, par, :], in0=tmpA, in1=tmpB,
                                        op=AX.add)

            for grp in range(8):
                zp = ps_zh.tile([64, 4, 256], F32, tag="zhp")
                for i in range(4):
                    cch = grp * 4 + i
                    lhs = Qst_c[:, cch, :]
                    nc.tensor.matmul(out=zp[0:32, i, :], lhsT=lhs, rhs=ghR2_s,
                                     start=True, stop=True)
                    nc.tensor.matmul(out=zp[32:64, i, :], lhsT=lhs, rhs=ghI2_s,
                                     start=True, stop=True, tile_position=(0, 32))
                nc.scalar.copy(
                    out=Zh2e[0:64, :, grp * 4:(grp + 1) * 4, :],
                    in_=zp.rearrange("p c (q l) -> p q c l", q=4))

    # ---------------- S3 + S4: main loop ----------------
    with tc.tile_pool(name="xqp", bufs=3) as xqp, \
         tc.tile_pool(name="h1sp", bufs=4) as h1sp, \
         tc.tile_pool(name="chain", bufs=2) as chain, \
         tc.tile_pool(name="sqjp", bufs=2) as sqjp, \
         tc.tile_pool(name="ps_zp", bufs=2, space="PSUM") as ps_zp, \
         tc.tile_pool(name="ps_h1", bufs=2, space="PSUM") as ps_h1:
        for fc in range(NFC):
            f0 = fc * FC
            base0 = BB + RS * (1 + 8 * fc)
            ZPa = ps_zp.tile([128, 1024], F32, tag="ZP")
            ZPb = ps_zp.tile([128, 1024], F32, tag="ZP")
            ZPh = [ZPa, ZPb]
            xqp2 = []
            for qp in range(2):
                xq = xqp.tile([128, FC], FP16, tag="xq")
                for j in range(2):
                    q = qp * 2 + j
                    nc.gpsimd.dma_start(out=xq[64 * j:64 * j + 64, :],
                                        in_=xbh_f[:, q * FQ + f0:q * FQ + f0 + FC])
                xqp2.append(xq)
            xqs = [xqp2[q // 2][64 * (q % 2):64 * (q % 2) + 64, :] for q in range(NQ)]
            h1ss = []
            for _q in range(NQ):
                h1s = h1sp.tile([128, FC], FP16, tag="h1s")
                h1ss.append(h1s)
            accD = chain.tile([128, 8 * RS], FP16, tag="accD")
            for hlf in range(2):
                s0 = hlf * 1024
                # mlp1 for this half (all 4 quarters) + gelu
                for q in range(NQ):
                    j = q % 2
                    hp = ps_h1.tile([128, 1024], F32, tag="h1p")
                    for s2 in range(0, 1024, 512):
                        nc.tensor.matmul(
                            out=hp[:, s2:s2 + 512],
                            lhsT=wm1T2_s[64 * j:64 * j + 64, :],
                            rhs=xqs[q][:, s0 + s2:s0 + s2 + 512],
                            start=True, stop=True, tile_position=(64 * j, 0))
                    nc.scalar.activation(out=h1ss[q][:, s0:s0 + 1024], in_=hp,
                                         func=AF.Gelu, bias=bm1_s, scale=1.0)
                ZP = ZPh[hlf]
                # local (start=True)
                for qp in range(2):
                    tp = (0, 64 * qp) if qp > 0 else None
                    for s in (s0, s0 + 512):
                        nc.tensor.matmul(
                            out=ZP[64 * qp:64 * qp + 64, s - s0:s - s0 + 512],
                            lhsT=wlocT2_s, rhs=xqp2[qp][:, s:s + 512],
                            start=True, stop=False, tile_position=tp,
                            skip_group_check=True)
                # PE dw taps (2 row-pairs per half)
                for t, (dy, dx) in enumerate(PE_TAPS):
                    d = dy * RS + dx
                    for mm in range(2):
                        m = hlf * 2 + mm
                        rst = base0 + 2 * RS * m + 2 + d
                        rhs = xbp[:, rst:rst + 2 * RS].rearrange(
                            "p (r z) -> p r z", r=2)[:, :, 0:256]
                        nc.tensor.matmul(out=ZP[:, mm * 512:mm * 512 + 512],
                                         lhsT=kdiag_s[:, t, :], rhs=rhs,
                                         start=False, stop=False,
                                         skip_group_check=True)
                # W-inverse y (+ bconst via 65th row)
                for i in range(4):
                    h0 = fc * 8 + hlf * 4 + i
                    nc.tensor.matmul(out=ZP[:, i * 256:i * 256 + 256],
                                     lhsT=Zh2e[:, :, :, h0], rhs=gw2e_s,
                                     start=False, stop=False,
                                     skip_group_check=True)
                # mlp2 (stop)
                for q in range(NQ):
                    tp = (0, 32 * q) if q > 0 else None
                    for s2 in range(0, 1024, 512):
                        nc.tensor.matmul(
                            out=ZP[32 * q:32 * q + 32, s2:s2 + 512],
                            lhsT=wm2T_s, rhs=h1ss[q][:, s0 + s2:s0 + s2 + 512],
                            start=False, stop=True, tile_position=tp,
                            skip_group_check=True)
                # DVE dw taps for this half (4 rows)
                a0 = hlf * 4 * RS
                for t, (dy, dx) in enumerate(DVE_TAPS):
                    d = dy * RS + dx
                    xs = xbp[:, base0 + a0 + d:base0 + a0 + d + 4 * RS]
                    nc.vector.scalar_tensor_tensor(
                        out=accD[:, a0:a0 + 4 * RS], in0=xs,
                        scalar=ktaps_s[:, t:t + 1],
                        in1=xs if t == 0 else accD[:, a0:a0 + 4 * RS],
                        op0=AX.mult,
                        op1=AX.bypass if t == 0 else AX.add)
                # merge: zbuf = ZP + accD (fp16), accum -> szc
                accDv = accD.rearrange("p (r z) -> p r z", r=8)
                nc.vector.scalar_tensor_tensor(
                    out=zbuf[:, f0 + s0:f0 + s0 + 1024],
                    in0=accDv[:, 4 * hlf:4 * hlf + 4, 2:258],
                    scalar=1.0, in1=ZP,
                    op0=AX.mult, op1=AX.add,
                    accum_out=szc[:, 2 * fc + hlf:2 * fc + hlf + 1])
            # --- sum(z^2) on ACT ---
            sqj = sqjp.tile([128, FC], FP16, tag="sqj")
            nc.scalar.activation(out=sqj, in_=zbuf[:, f0:f0 + FC], func=AF.Square,
                                 accum_out=sqc[:, fc:fc + 1])

    # ---------------- S5: stats ----------------
    st = ctx.enter_context(tc.tile_pool(name="stats", bufs=1))
    with tc.tile_pool(name="ps_st", bufs=1, space="PSUM") as ps_st:
        sums = st.tile([128, 2], F32, tag="sums")
        nc.vector.tensor_reduce(out=sums[:, 0:1], in_=szc,
                                axis=mybir.AxisListType.X, op=AX.add)
        nc.vector.tensor_reduce(out=sums[:, 1:2], in_=sqc,
                                axis=mybir.AxisListType.X, op=AX.add)
        sp = ps_st.tile([32, 2], F32, tag="sp")
        nc.tensor.matmul(out=sp, lhsT=qones_s, rhs=sums, start=True, stop=True)
        mu = st.tile([32, 1], F32, tag="mu")
        negmu = st.tile([32, 1], F32, tag="negmu")
        ex2 = st.tile([32, 1], F32, tag="ex2")
        var = st.tile([32, 1], F32, tag="var")
        s12 = st.tile([32, 2], F32, tag="s12")
        inv_n = 1.0 / float(HW)
        nc.vector.tensor_scalar(out=mu, in0=sp[:, 0:1], scalar1=inv_n,
                                scalar2=None, op0=AX.mult)
        nc.vector.tensor_scalar(out=negmu, in0=sp[:, 0:1], scalar1=-inv_n,
                                scalar2=None, op0=AX.mult)
        nc.vector.tensor_scalar(out=ex2, in0=sp[:, 1:2], scalar1=inv_n,
                                scalar2=None, op0=AX.mult)
        nc.vector.scalar_tensor_tensor(out=var, in0=mu, scalar=negmu, in1=ex2,
                                       op0=AX.mult, op1=AX.add)
        epst = st.tile([32, 1], F32, tag="epst")
        nc.vector.memset(epst, 1e-5)
        nc.scalar.activation(out=var, in_=var, func=AF.Sqrt, bias=epst, scale=1.0)
        nc.vector.reciprocal(out=var, in_=var)
        nc.vector.tensor_tensor(out=s12[:, 0:1], in0=var, in1=gam_s, op=AX.mult)
        nc.vector.tensor_scalar(out=negmu, in0=mu, scalar1=-1.0,
                                scalar2=None, op0=AX.mult)
        nc.vector.scalar_tensor_tensor(out=s12[:, 1:2], in0=s12[:, 0:1],
                                       scalar=negmu, in1=bet_s,
                                       op0=AX.mult, op1=AX.add)
        spb = ps_st.tile([128, 2], F32, tag="spb")
        nc.tensor.matmul(out=spb, lhsT=qonesT_s, rhs=s12, start=True, stop=True)
        s12s = st.tile([128, 2], F32, tag="s12s")
        nc.vector.tensor_copy(out=s12s, in_=spb)

    # ---------------- S6: fc pairs, gelu + residual, split queues ----------
    with tc.tile_pool(name="sw2", bufs=3) as sw2:
        for fc in range(NFC):
            f0 = fc * FC
            base0 = BB + RS * (1 + 8 * fc)
            g = sw2.tile([128, FC], FP16, tag="g")
            nc.scalar.activation(out=g, in_=zbuf[:, f0:f0 + FC], func=AF.Gelu,
                                 bias=s12s[:, 1:2], scale=s12s[:, 0:1])
            xwin = xbp[:, base0:base0 + 8 * RS].rearrange(
                "p (r z) -> p r z", r=8)[:, :, 2:258]
            gv = g.rearrange("p (r z) -> p r z", r=8)
            ob = sw2.tile([128, FC], F32, tag="ob")
            obv = ob.rearrange("p (r z) -> p r z", r=8)
            nc.vector.tensor_tensor(out=obv, in0=gv, in1=xwin, op=AX.add)
            for q in range(NQ):
                eng = nc.sync if q % 2 == 0 else nc.gpsimd
                eng.dma_start(out=out_f[:, q * FQ + f0:q * FQ + f0 + FC],
                              in_=ob[32 * q:32 * q + 32, :])


_PROGRAM = None


def kernel(**inputs):
    global _PROGRAM
    in_maps = _per_core_inputs(inputs)
    if _PROGRAM is None:
        _PROGRAM = _build_program()
    res = run_bass_kernel_spmd(_PROGRAM, in_maps, list(range(N_CORES)))
    x = np.asarray(inputs["x"], np.float32)
    out = np.empty_like(x)
    for core in range(N_CORES):
        b, half = core // 2, core % 2
        out[b, half * 32:half * 32 + 32] = res.results[core]["outp"]
    return out
